# revision 22
# baseline (speedup 1.0000x reference)
"""Trainium2 Bass kernel for nn_Attention_89172110999574.

Strategy (8 NeuronCores, data parallel - 1 batch element per core), v4:
  - All matmul operands bf16 (1 cyc/row on PE); PSUM stays f32.
  - Scores computed TRANSPOSED (ST[j,i] = k_j . q_i); softmax scale folded
    into Wq at load.
  - Relative-position bias is block-Toeplitz; two per-head strip tables are
    built once via a handful of large-elem DMAs (DRAM bounce for the
    partition reshuffle):
      * ms  (bf16):  exp(bias) factors for the exact-exp path (jt 0-3)
      * msa (int16): bias*A + B Schraudolph addends for the fast path
    exp(s+b) = exp(s)*exp(b) on the A path (ACT exp + DVE/Pool
    tensor_tensor multiply at 2x bf16 rate), and
    bf16_bits(exp(s+b)) ~= s*A + (b*A + B) on the V path - a single DVE
    op (Schraudolph's trick in bf16, max ~3% weight ripple, cancels in
    the softmax ratio; validated 4e-3 end-to-end).
  - attn@V uses exp-scores as stationary -> [i, dv+1] tiles with the
    softmax denominator in the last column.  Each PSUM accumulation group
    runs as 8 consecutive matmuls (interleaved groups in one bank
    misaccumulate on hardware).
  - normalize+gelu fused: ACT Gelu with per-partition scale = 1/den
    (reciprocals batched 4-wide on DVE) reading straight from PSUM.
  - BatchNorm affine folded into Wo/bias rows; per-i-tile tail:
    transpose via PE, final contraction, store.
"""

import os
import sys

import numpy as np

for _p in ("/opt/trn_rl_repo", "/root/.axon_site/_ro/trn_rl_repo"):
    if os.path.isdir(_p) and _p not in sys.path:
        sys.path.insert(0, _p)

import concourse.bass as bass
import concourse.tile as tile
from concourse import mybir
from concourse.bass_utils import run_bass_kernel_spmd
from concourse.masks import make_identity

N = 1024          # tokens per batch (32*32)
D = 256           # model dim
H = 8             # heads
DK = 32           # head dim (qk)
DV = 64           # head dim (v)
DOUT = 256        # output dim
NCORES = 8
FM = 32           # fmap
SCALE = float(DK) ** -0.5          # 1/sqrt(32)
BN_C = float(1.0 / np.sqrt(1.0 + 1e-5))
SCH_A = float(2 ** 7 / np.log(2.0))        # Schraudolph bf16 scale
SCH_B = float(127 * 2 ** 7 - 7.4 + 0.5)    # bias - minimax + trunc comp
F32 = mybir.dt.float32
BF16 = mybir.dt.bfloat16
I16 = mybir.dt.int16
AF = mybir.ActivationFunctionType
ALU = mybir.AluOpType

# jt 0..3: exact exp on ACT (+ bias multiply); jt 4..7: Schraudolph on DVE
N_A = 4
POOL_BIAS_JT = (0, 1, 2)       # A-tiles whose bias multiply runs on Pool
# strip-table u ranges (u0 = 31-4*jt, slice [u0, u0+32))
MS_U0, MS_UN = 19, 44          # A tiles: u in [19, 62]
MSA_U0, MSA_UN = 3, 44         # V tiles: u in [3, 46]


def build_nc():
    nc = bass.Bass("TRN2", target_bir_lowering=False, debug=False)

    x = nc.dram_tensor("x", [N, D], F32, kind="ExternalInput").ap()
    wq = nc.dram_tensor("wq", [D, H * DK], F32, kind="ExternalInput").ap()
    wk = nc.dram_tensor("wk", [D, H * DK], F32, kind="ExternalInput").ap()
    wv = nc.dram_tensor("wv", [D, H * DV], F32, kind="ExternalInput").ap()
    wo = nc.dram_tensor("wo", [H * DV, DOUT], F32, kind="ExternalInput").ap()
    pe = nc.dram_tensor("pe", [N, H], F32, kind="ExternalInput").ap()
    bo = nc.dram_tensor("bo", [DOUT], F32, kind="ExternalInput").ap()
    gam = nc.dram_tensor("gam", [DOUT], F32, kind="ExternalInput").ap()
    bet = nc.dram_tensor("bet", [DOUT], F32, kind="ExternalInput").ap()
    out = nc.dram_tensor("out", [N, DOUT], F32, kind="ExternalOutput").ap()

    # DRAM bounce buffers for the strip tables: [a, h, s] flipped rows
    wrowd = nc.dram_tensor("wrowd", [32, 8, 63], BF16).ap()
    wrowda = nc.dram_tensor("wrowda", [32, 8, 63], I16).ap()

    with tile.TileContext(nc) as tc:
        with (
            tc.tile_pool(name="const", bufs=1) as constp,
            tc.tile_pool(name="big", bufs=1) as bigp,
            tc.tile_pool(name="xin", bufs=3) as xinp,
            tc.tile_pool(name="exps", bufs=4) as expp,
            tc.tile_pool(name="esb", bufs=1) as esbp,
            tc.tile_pool(name="small", bufs=2) as smallp,
            tc.tile_pool(name="yout", bufs=3) as youtp,
            tc.tile_pool(name="ps1", bufs=2, space="PSUM") as ps1p,
            tc.tile_pool(name="st", bufs=4, space="PSUM") as ps2p,
            tc.tile_pool(name="po", bufs=1, space="PSUM") as pop,
        ):
            # ------------- input / weight DMAs first (fabric order) -----
            xa = bigp.tile([128, 8, 256], F32)
            for c in range(2):
                nc.sync.dma_start(
                    out=xa[:, 4 * c:4 * (c + 1), :],
                    in_=bass.AP(tensor=x.tensor, offset=4 * c * 128 * 256,
                                ap=[[256, 128], [128 * 256, 4], [1, 256]]))
            wstg_q = xinp.tile([128, 2, 256], F32, tag="wstg2", bufs=2)
            nc.scalar.dma_start(
                out=wstg_q,
                in_=bass.AP(tensor=wq.tensor, offset=0,
                            ap=[[256, 128], [128 * 256, 2], [1, 256]]))
            wstg_k = xinp.tile([128, 2, 256], F32, tag="wstg2", bufs=2)
            nc.scalar.dma_start(
                out=wstg_k,
                in_=bass.AP(tensor=wk.tensor, offset=0,
                            ap=[[256, 128], [128 * 256, 2], [1, 256]]))
            wstg_v = xinp.tile([128, 2, 512], F32, tag="wstgv", bufs=1)
            nc.scalar.dma_start(
                out=wstg_v,
                in_=bass.AP(tensor=wv.tensor, offset=0,
                            ap=[[512, 128], [128 * 512, 2], [1, 512]]))
            e_sb = smallp.tile([32, 32, 8], F32, tag="e_sb")
            nc.sync.dma_start(
                out=e_sb,
                in_=bass.AP(tensor=pe.tensor, offset=0,
                            ap=[[32 * H, 32], [H, 32], [1, 32 * H // 32]]),
            )

            # ---------------- constants -------------------------------
            identb = constp.tile([128, 128], BF16)
            make_identity(nc, identb)
            identf = constp.tile([128, 128], F32)
            nc.gpsimd.tensor_copy(identf, identb)

            # ---------------- strip tables ----------------------------
            # 2a) mult table: ee = exp(pe/scale) bf16
            ee = smallp.tile([32, 32, 8], BF16, tag="ee")
            nc.scalar.activation(ee, e_sb, AF.Exp, scale=1.0 / SCALE)
            # 2b) additive Schraudolph table: eea = pe/scale*A + B int16
            eea = smallp.tile([32, 32, 8], I16, tag="eea")
            nc.scalar.activation(eea, e_sb, AF.Copy,
                                 scale=SCH_A / SCALE, bias=SCH_B)
            # 3) s-flip both: wrow[a, h, s] = tab[a, |s-31|, h]
            wrow = smallp.tile([32, 8, 63], BF16, tag="wrow")
            wrowa = smallp.tile([32, 8, 63], I16, tag="wrowa")
            for wdst, wsrc in ((wrow, ee), (wrowa, eea)):
                nc.gpsimd.tensor_copy(
                    wdst[:, :, 0:31],
                    bass.AP(tensor=wsrc.tensor, offset=wsrc.offset + 31 * 8,
                            ap=[wsrc.ap[0], [1, 8], [-8, 31]]),
                )
                nc.gpsimd.tensor_copy(
                    wdst[:, :, 31:63],
                    bass.AP(tensor=wsrc.tensor, offset=wsrc.offset,
                            ap=[wsrc.ap[0], [1, 8], [8, 32]]),
                )
            # 4) DRAM bounce + gather (positive strides; wrow palindromic
            #    in s, so the gathered ci axis comes out reversed)
            nc.sync.dma_start(out=wrowd, in_=wrow)
            nc.scalar.dma_start(out=wrowda, in_=wrowa)
            # ms[(g,cj), u-U0, h, ci'] = tab_h[|u-31-g|, |ci-cj|]
            ms = bigp.tile([128, MS_UN, H, 32], BF16)
            msa = bigp.tile([128, MSA_UN, H, 32], I16)
            with tc.tile_pool(name="uw", bufs=1) as uwp:
                uwsb = uwp.tile([32, 32, H, 32], BF16)
                uwsba = uwp.tile([32, 32, H, 32], I16)
                nc.sync.dma_start(
                    out=uwsb,
                    in_=bass.AP(tensor=wrowd.tensor, offset=0,
                                ap=[[1, 32], [504, 32], [63, 8], [1, 32]]),
                )
                nc.scalar.dma_start(
                    out=uwsba,
                    in_=bass.AP(tensor=wrowda.tensor, offset=0,
                                ap=[[1, 32], [504, 32], [63, 8], [1, 32]]),
                )
                # 5) u-expansion: dst[(g,cj), u] = uwsb[cj, |u-31-g|]
                engs = (nc.sync, nc.scalar, nc.gpsimd)
                ei = 0
                for src, dst, u0t, un in ((uwsb, ms, MS_U0, MS_UN),
                                          (uwsba, msa, MSA_U0, MSA_UN)):
                    for g in range(4):
                        # upper: u in [31+g, u0t+un), a = u-31-g ascending
                        ua, ub = 31 + g, u0t + un
                        engs[ei % 3].dma_start(
                            out=dst[32 * g:32 * (g + 1), ua - u0t:ub - u0t,
                                    :, :],
                            in_=src[:, 0:ub - ua, :, :],
                        )
                        ei += 1
                        # lower: u in [u0t, 31+g), a = 31+g-u descending
                        la, lb = u0t, 31 + g
                        amax = 31 + g - u0t
                        engs[ei % 3].dma_start(
                            out=dst[32 * g:32 * (g + 1), 0:lb - la, :, :],
                            in_=bass.AP(tensor=src.tensor,
                                        offset=src.offset + amax * 256,
                                        ap=[src.ap[0], [-256, lb - la],
                                            [1, 256]]),
                        )
                        ei += 1

            # ---------------- BN affine rows ---------------------------
            g2b = constp.tile([128, DOUT], F32)
            b2b = constp.tile([128, DOUT], F32)
            tmpb = constp.tile([128, DOUT], F32)
            nc.sync.dma_start(
                out=g2b, in_=bass.AP(tensor=gam.tensor, offset=0,
                                     ap=[[0, 128], [1, DOUT]]))
            nc.sync.dma_start(
                out=b2b, in_=bass.AP(tensor=bet.tensor, offset=0,
                                     ap=[[0, 128], [1, DOUT]]))
            nc.sync.dma_start(
                out=tmpb, in_=bass.AP(tensor=bo.tensor, offset=0,
                                      ap=[[0, 128], [1, DOUT]]))
            wstg_o = xinp.tile([128, 4, 256], F32, tag="wstgo", bufs=1)
            nc.sync.dma_start(
                out=wstg_o,
                in_=bass.AP(tensor=wo.tensor, offset=0,
                            ap=[[256, 128], [128 * 256, 4], [1, 256]]))
            nc.scalar.mul(g2b, g2b, BN_C)
            nc.vector.tensor_mul(tmpb, tmpb, g2b)
            nc.vector.tensor_add(b2b, b2b, tmpb)

            # ---------------- weights (convert to bf16) ----------------
            wq_sb = constp.tile([128, 2, 256], BF16)
            wk_sb = constp.tile([128, 2, 256], BF16)
            wv_sb = constp.tile([128, 2, 512], BF16)
            wo_sb = constp.tile([128, 4, 256], BF16)
            nc.scalar.mul(wq_sb, wstg_q, SCALE)   # fold softmax scale
            nc.gpsimd.tensor_copy(wk_sb, wstg_k)
            nc.gpsimd.tensor_copy(wv_sb, wstg_v)
            # fold BN gamma*c into Wo columns (broadcast g2b over kt)
            nc.vector.tensor_mul(
                wo_sb, wstg_o,
                bass.AP(tensor=g2b.tensor, offset=g2b.offset,
                        ap=[g2b.ap[0], [0, 4], [1, 256]]))

            def _copy(i, dst, src):
                e = (nc.scalar.copy, nc.vector.tensor_copy)[i % 2]
                e(dst, src)

            # ---------------- phase A: x -> xT (bf16) ------------------
            xT = bigp.tile([128, 2, N], BF16)
            for nt in range(8):
                for dt in range(2):
                    pst = ps1p.tile([128, 512], F32, tag="ps1")
                    nc.tensor.transpose(pst[:, 0:128],
                                        xa[:, nt, 128 * dt:128 * (dt + 1)],
                                        identf)
                    _copy(nt + dt, xT[:, dt, 128 * nt:128 * (nt + 1)],
                          pst[:, 0:128])

            # ---------------- phase B: QKV proj ------------------------
            qT = bigp.tile([128, 2, N], BF16)
            kT = bigp.tile([128, 2, N], BF16)
            va = bigp.tile([128, 8, H, 65], BF16)
            nc.scalar.activation(va[:, :, :, 64:65], identb[:, 0:64],
                                 AF.Copy, bias=1.0, scale=0.0)
            ci_ = 0

            def _qk(mt):
                nonlocal ci_
                for dst_sb, w_sb in ((qT, wq_sb), (kT, wk_sb)):
                    for ic in range(2):
                        ps = ps1p.tile([128, 512], F32, tag="ps1")
                        for kt in range(2):
                            nc.tensor.matmul(
                                ps,
                                w_sb[:, kt, 128 * mt:128 * (mt + 1)],
                                xT[:, kt, 512 * ic:512 * (ic + 1)],
                                start=(kt == 0), stop=(kt == 1),
                            )
                        _copy(ci_, dst_sb[:, mt, 512 * ic:512 * (ic + 1)], ps)
                        ci_ += 1

            _qk(0)   # heads 0-3 only need mt 0 - lets phase C start early
            for jt in range(8):
                ps = ps1p.tile([128, 512], F32, tag="ps1")
                for kt in range(2):
                    nc.tensor.matmul(
                        ps,
                        xT[:, kt, 128 * jt:128 * (jt + 1)],
                        wv_sb[:, kt, :],
                        start=(kt == 0), stop=(kt == 1),
                    )
                psr = ps.rearrange("p (h v) -> p h v", v=64)
                _copy(ci_, va[:, jt, :, 0:64], psr)
                ci_ += 1
            _qk(1)

            # ---------------- phase C: attention -----------------------
            # g_all[i-part, it, h, dv] collects gelu(attn/den), bf16
            g_all = bigp.tile([128, 8, H, DV], BF16)
            gtt = bigp.tile([128, 4, 8, 128], BF16)

            def _tail_transpose(blk):
                # gtt[:, blk, it, :] = g_all[:, it, 2blk:2blk+2, :]^T
                for it in range(8):
                    pst = ps1p.tile([128, 512], F32, tag="ps1")
                    pstb = pst[:, 0:64].bitcast(BF16)
                    nc.tensor.transpose(
                        pstb,
                        g_all[:, it, 2 * blk:2 * blk + 2, :], identb)
                    nc.vector.tensor_copy(gtt[:, blk, it, :], pstb)

            def _attnv_group(h, it, esbbs, pos):
                for jt in range(8):
                    nc.tensor.matmul(
                        pos[it // 4][:, it % 4, :],
                        esbbs[jt][:, 128 * it:128 * (it + 1)],
                        va[:, jt, h, :],
                        start=(jt == 0), stop=(jt == 7),
                    )

            def _norm_gelu(h, half, pos, rcp):
                den = pos[half][:, :, 64:65]
                nc.vector.reciprocal(
                    rcp[:, 4 * half:4 * half + 4],
                    bass.AP(tensor=den.tensor, offset=den.offset,
                            ap=[den.ap[0], [65, 4]]))
                for it in range(4 * half, 4 * half + 4):
                    nc.scalar.activation(
                        g_all[:, it, h, :],
                        pos[half][:, it % 4, 0:64],
                        AF.Gelu, scale=rcp[:, it:it + 1])

            # attnV/normalize are software-pipelined one head behind the
            # score/exp stream so the attnV matmul groups fill PE stalls
            # between score matmuls (keeps PE p-state ramped).
            prev = None
            for h in range(H):
                mtk = h // 4
                pb = 32 * (h % 4)
                po0 = pop.tile([128, 4, 65], F32, tag="po0")
                po1 = pop.tile([128, 4, 65], F32, tag="po1")
                pos = (po0, po1)
                esbbs = [None] * 8
                for step, jt in enumerate((0, 4, 1, 5, 2, 6, 3, 7)):
                    u0 = 31 - 4 * jt
                    if jt < N_A:
                        esb = esbp.tile([128, 1024], BF16, tag="esb", bufs=8)
                    else:
                        esb = esbp.tile([128, 1024], I16, tag="esbi", bufs=8)
                    for ic in range(2):
                        ps = ps2p.tile([128, 512], F32, tag="st")
                        nc.tensor.matmul(
                            ps,
                            kT[pb:pb + 32, mtk, 128 * jt:128 * (jt + 1)],
                            qT[pb:pb + 32, mtk, 512 * ic:512 * (ic + 1)],
                            start=True, stop=True,
                            tile_position=(pb, 0),
                        )
                        if jt < N_A:
                            # exact exp on ACT per half
                            nc.scalar.activation(
                                esb[:, 512 * ic:512 * (ic + 1)], ps, AF.Exp)
                        else:
                            # fused Schraudolph exp+bias per half on DVE
                            uh = u0 - MSA_U0 + 16 * ic
                            msl = msa[:, uh:uh + 16, h, :]
                            msr = bass.AP(tensor=msl.tensor,
                                          offset=msl.offset + 31,
                                          ap=[msl.ap[0], msl.ap[1],
                                              [-1, 32]])
                            nc.vector.scalar_tensor_tensor(
                                esb[:, 512 * ic:512 * (ic + 1)], ps,
                                SCH_A, msr, ALU.mult, ALU.add)
                    if jt < N_A:
                        # bias multiply full-tile, in place (DVE 2x / Pool)
                        msl = ms[:, u0 - MS_U0:u0 - MS_U0 + 32, h, :]
                        msr = bass.AP(tensor=msl.tensor,
                                      offset=msl.offset + 31,
                                      ap=[msl.ap[0], msl.ap[1], [-1, 32]])
                        beng = nc.gpsimd if jt in POOL_BIAS_JT else nc.vector
                        esb2 = esbp.tile([128, 1024], BF16, tag="esb2",
                                         bufs=8)
                        beng.tensor_tensor(esb2, esb, msr, ALU.mult)
                        esbbs[jt] = esb2
                    else:
                        esbbs[jt] = esb.bitcast(BF16)
                    if prev is not None:
                        _attnv_group(h - 1, step, prev[0], prev[1])
                        if step == 3:
                            _norm_gelu(h - 1, 0, prev[1], prev[2])
                        elif step == 7:
                            _norm_gelu(h - 1, 1, prev[1], prev[2])
                            if h % 2 == 0:
                                _tail_transpose(h // 2 - 1)
                rcp = smallp.tile([128, 8], F32, tag="rcp", bufs=2)
                prev = (esbbs, pos, rcp)
            # drain last head: attnV groups + normalize + transposes
            for it in range(8):
                _attnv_group(7, it, prev[0], prev[1])
                if it == 3:
                    _norm_gelu(7, 0, prev[1], prev[2])
            _norm_gelu(7, 1, prev[1], prev[2])
            _tail_transpose(3)

            # ---------------- tail: out proj ---------------------------
            for it in range(8):
                ps = ps1p.tile([128, 512], F32, tag="ps1")
                for kt in range(4):
                    nc.tensor.matmul(
                        ps[:, 0:256],
                        gtt[:, kt, it, :],
                        wo_sb[:, kt, :],
                        start=(kt == 0), stop=(kt == 3),
                    )
                yt = youtp.tile([128, DOUT], F32, tag="yt")
                nc.vector.tensor_add(yt, ps[:, 0:256], b2b)
                eng = nc.sync if it % 2 == 0 else nc.scalar
                eng.dma_start(out=out[128 * it:128 * (it + 1), :], in_=yt)

    _split_excess_waits(nc)
    return nc


def _split_excess_waits(nc):
    """walrus rejects >1 sem-wait per instruction ("Too many sync wait
    commands"); unroll extras into a chain of single-wait same-engine
    NoOps directly before the instruction."""
    ctr = 0
    for fn in nc.m.functions:
        for blk in fn.blocks:
            out = []
            for inst in blk.instructions:
                si = inst.sync_info
                if si is not None and len(si.on_wait) > 1:
                    for w in si.on_wait[:-1]:
                        nop = mybir.InstNoOp(name=f"waitnop-{ctr}")
                        ctr += 1
                        nop.engine = inst.engine
                        nop.sync_info = mybir.SyncInfo(
                            on_wait=[w], on_update=[])
                        out.append(nop)
                    inst.sync_info = mybir.SyncInfo(
                        on_wait=[si.on_wait[-1]], on_update=list(si.on_update))
                out.append(inst)
            blk.instructions = out


_NC_CACHE = None


def kernel(**inputs) -> np.ndarray:
    global _NC_CACHE
    x = np.ascontiguousarray(inputs["x"], dtype=np.float32)        # (8,32,32,256)
    shared = {
        "wq": np.ascontiguousarray(inputs["Wq"], dtype=np.float32),
        "wk": np.ascontiguousarray(inputs["Wk"], dtype=np.float32),
        "wv": np.ascontiguousarray(inputs["Wv"], dtype=np.float32),
        "wo": np.ascontiguousarray(inputs["Wo"], dtype=np.float32),
        "pe": np.ascontiguousarray(inputs["pos_emb"], dtype=np.float32),
        "bo": np.ascontiguousarray(inputs["bo"], dtype=np.float32),
        "gam": np.ascontiguousarray(inputs["gamma"], dtype=np.float32),
        "bet": np.ascontiguousarray(inputs["beta"], dtype=np.float32),
    }
    in_maps = []
    for c in range(NCORES):
        m = dict(shared)
        m["x"] = np.ascontiguousarray(x[c].reshape(N, D))
        in_maps.append(m)

    if _NC_CACHE is None:
        _NC_CACHE = build_nc()
    res = run_bass_kernel_spmd(_NC_CACHE, in_maps, core_ids=list(range(NCORES)))
    outs = [res.results[c]["out"].reshape(FM, FM, DOUT) for c in range(NCORES)]
    return np.stack(outs, axis=0)


if __name__ == "__main__":
    build_nc()
    print("build ok")


# revision 24
# speedup vs baseline: 1.0259x; 1.0259x over previous
"""Trainium2 Bass kernel for nn_Attention_89172110999574.

Strategy (8 NeuronCores, data parallel - 1 batch element per core), v4:
  - All matmul operands bf16 (1 cyc/row on PE); PSUM stays f32.
  - Scores computed TRANSPOSED (ST[j,i] = k_j . q_i); softmax scale folded
    into Wq at load.
  - Relative-position bias is block-Toeplitz; two per-head strip tables are
    built once via a handful of large-elem DMAs (DRAM bounce for the
    partition reshuffle):
      * ms  (bf16):  exp(bias) factors for the exact-exp path (jt 0-3)
      * msa (int16): bias*A + B Schraudolph addends for the fast path
    exp(s+b) = exp(s)*exp(b) on the A path (ACT exp + DVE/Pool
    tensor_tensor multiply at 2x bf16 rate), and
    bf16_bits(exp(s+b)) ~= s*A + (b*A + B) on the V path - a single DVE
    op (Schraudolph's trick in bf16, max ~3% weight ripple, cancels in
    the softmax ratio; validated 4e-3 end-to-end).
  - attn@V uses exp-scores as stationary -> [i, dv+1] tiles with the
    softmax denominator in the last column.  Each PSUM accumulation group
    runs as 8 consecutive matmuls (interleaved groups in one bank
    misaccumulate on hardware).
  - normalize+gelu fused: ACT Gelu with per-partition scale = 1/den
    (reciprocals batched 4-wide on DVE) reading straight from PSUM.
  - BatchNorm affine folded into Wo/bias rows; per-i-tile tail:
    transpose via PE, final contraction, store.
"""

import os
import sys

import numpy as np

for _p in ("/opt/trn_rl_repo", "/root/.axon_site/_ro/trn_rl_repo"):
    if os.path.isdir(_p) and _p not in sys.path:
        sys.path.insert(0, _p)

import concourse.bass as bass
import concourse.tile as tile
from concourse import mybir
from concourse.bass_utils import run_bass_kernel_spmd
from concourse.masks import make_identity

N = 1024          # tokens per batch (32*32)
D = 256           # model dim
H = 8             # heads
DK = 32           # head dim (qk)
DV = 64           # head dim (v)
DOUT = 256        # output dim
NCORES = 8
FM = 32           # fmap
SCALE = float(DK) ** -0.5          # 1/sqrt(32)
BN_C = float(1.0 / np.sqrt(1.0 + 1e-5))
SCH_A = float(2 ** 7 / np.log(2.0))        # Schraudolph bf16 scale
SCH_B = float(127 * 2 ** 7 - 7.4 + 0.5)    # bias - minimax + trunc comp
F32 = mybir.dt.float32
BF16 = mybir.dt.bfloat16
I16 = mybir.dt.int16
AF = mybir.ActivationFunctionType
ALU = mybir.AluOpType

# jt 0..3: exact exp on ACT (+ bias multiply); jt 4..7: Schraudolph on DVE
N_A = 4
POOL_BIAS_JT = (1, 2)          # A-tiles whose bias multiply runs on Pool
# strip-table u ranges (u0 = 31-4*jt, slice [u0, u0+32))
MS_U0, MS_UN = 19, 44          # A tiles: u in [19, 62]
MSA_U0, MSA_UN = 3, 44         # V tiles: u in [3, 46]


def build_nc():
    nc = bass.Bass("TRN2", target_bir_lowering=False, debug=False)

    x = nc.dram_tensor("x", [N, D], F32, kind="ExternalInput").ap()
    wq = nc.dram_tensor("wq", [D, H * DK], F32, kind="ExternalInput").ap()
    wk = nc.dram_tensor("wk", [D, H * DK], F32, kind="ExternalInput").ap()
    wv = nc.dram_tensor("wv", [D, H * DV], F32, kind="ExternalInput").ap()
    wo = nc.dram_tensor("wo", [H * DV, DOUT], F32, kind="ExternalInput").ap()
    pe = nc.dram_tensor("pe", [N, H], F32, kind="ExternalInput").ap()
    bo = nc.dram_tensor("bo", [DOUT], F32, kind="ExternalInput").ap()
    gam = nc.dram_tensor("gam", [DOUT], F32, kind="ExternalInput").ap()
    bet = nc.dram_tensor("bet", [DOUT], F32, kind="ExternalInput").ap()
    out = nc.dram_tensor("out", [N, DOUT], F32, kind="ExternalOutput").ap()

    # DRAM bounce buffers for the strip tables: [a, h, s] flipped rows
    wrowd = nc.dram_tensor("wrowd", [32, 8, 63], BF16).ap()
    wrowda = nc.dram_tensor("wrowda", [32, 8, 63], I16).ap()

    with tile.TileContext(nc) as tc:
        with (
            tc.tile_pool(name="const", bufs=1) as constp,
            tc.tile_pool(name="big", bufs=1) as bigp,
            tc.tile_pool(name="xin", bufs=3) as xinp,
            tc.tile_pool(name="exps", bufs=4) as expp,
            tc.tile_pool(name="esb", bufs=1) as esbp,
            tc.tile_pool(name="small", bufs=2) as smallp,
            tc.tile_pool(name="yout", bufs=3) as youtp,
            tc.tile_pool(name="ps1", bufs=2, space="PSUM") as ps1p,
            tc.tile_pool(name="st", bufs=2, space="PSUM") as ps2p,
            tc.tile_pool(name="po", bufs=1, space="PSUM") as pop,
        ):
            # ------------- input / weight DMAs first (fabric order) -----
            xa = bigp.tile([128, 8, 256], F32)
            for c in range(2):
                nc.sync.dma_start(
                    out=xa[:, 4 * c:4 * (c + 1), :],
                    in_=bass.AP(tensor=x.tensor, offset=4 * c * 128 * 256,
                                ap=[[256, 128], [128 * 256, 4], [1, 256]]))
            wstg_q = xinp.tile([128, 2, 256], F32, tag="wstg2", bufs=2)
            nc.scalar.dma_start(
                out=wstg_q,
                in_=bass.AP(tensor=wq.tensor, offset=0,
                            ap=[[256, 128], [128 * 256, 2], [1, 256]]))
            wstg_k = xinp.tile([128, 2, 256], F32, tag="wstg2", bufs=2)
            nc.scalar.dma_start(
                out=wstg_k,
                in_=bass.AP(tensor=wk.tensor, offset=0,
                            ap=[[256, 128], [128 * 256, 2], [1, 256]]))
            wstg_v = xinp.tile([128, 2, 512], F32, tag="wstgv", bufs=1)
            nc.scalar.dma_start(
                out=wstg_v,
                in_=bass.AP(tensor=wv.tensor, offset=0,
                            ap=[[512, 128], [128 * 512, 2], [1, 512]]))
            e_sb = smallp.tile([32, 32, 8], F32, tag="e_sb")
            nc.sync.dma_start(
                out=e_sb,
                in_=bass.AP(tensor=pe.tensor, offset=0,
                            ap=[[32 * H, 32], [H, 32], [1, 32 * H // 32]]),
            )

            # ---------------- constants -------------------------------
            identb = constp.tile([128, 128], BF16)
            make_identity(nc, identb)
            identf = constp.tile([128, 128], F32)
            nc.gpsimd.tensor_copy(identf, identb)

            # ---------------- strip tables ----------------------------
            # 2a) mult table: ee = exp(pe/scale) bf16
            ee = smallp.tile([32, 32, 8], BF16, tag="ee")
            nc.scalar.activation(ee, e_sb, AF.Exp, scale=1.0 / SCALE)
            # 2b) additive Schraudolph table: eea = pe/scale*A + B int16
            eea = smallp.tile([32, 32, 8], I16, tag="eea")
            nc.scalar.activation(eea, e_sb, AF.Copy,
                                 scale=SCH_A / SCALE, bias=SCH_B)
            # 3) s-flip both: wrow[a, h, s] = tab[a, |s-31|, h]
            wrow = smallp.tile([32, 8, 63], BF16, tag="wrow")
            wrowa = smallp.tile([32, 8, 63], I16, tag="wrowa")
            for wdst, wsrc in ((wrow, ee), (wrowa, eea)):
                nc.gpsimd.tensor_copy(
                    wdst[:, :, 0:31],
                    bass.AP(tensor=wsrc.tensor, offset=wsrc.offset + 31 * 8,
                            ap=[wsrc.ap[0], [1, 8], [-8, 31]]),
                )
                nc.gpsimd.tensor_copy(
                    wdst[:, :, 31:63],
                    bass.AP(tensor=wsrc.tensor, offset=wsrc.offset,
                            ap=[wsrc.ap[0], [1, 8], [8, 32]]),
                )
            # 4) DRAM bounce + gather (positive strides; wrow palindromic
            #    in s, so the gathered ci axis comes out reversed)
            nc.sync.dma_start(out=wrowd, in_=wrow)
            nc.scalar.dma_start(out=wrowda, in_=wrowa)
            # ms[(g,cj), u-U0, h, ci'] = tab_h[|u-31-g|, |ci-cj|]
            ms = bigp.tile([128, MS_UN, H, 32], BF16)
            msa = bigp.tile([128, MSA_UN, H, 32], I16)
            with tc.tile_pool(name="uw", bufs=1) as uwp:
                uwsb = uwp.tile([32, 32, H, 32], BF16)
                uwsba = uwp.tile([32, 32, H, 32], I16)
                nc.sync.dma_start(
                    out=uwsb,
                    in_=bass.AP(tensor=wrowd.tensor, offset=0,
                                ap=[[1, 32], [504, 32], [63, 8], [1, 32]]),
                )
                nc.scalar.dma_start(
                    out=uwsba,
                    in_=bass.AP(tensor=wrowda.tensor, offset=0,
                                ap=[[1, 32], [504, 32], [63, 8], [1, 32]]),
                )
                # 5) u-expansion: dst[(g,cj), u] = uwsb[cj, |u-31-g|]
                engs = (nc.sync, nc.scalar, nc.gpsimd)
                ei = 0
                for src, dst, u0t, un in ((uwsb, ms, MS_U0, MS_UN),
                                          (uwsba, msa, MSA_U0, MSA_UN)):
                    for g in range(4):
                        # upper: u in [31+g, u0t+un), a = u-31-g ascending
                        ua, ub = 31 + g, u0t + un
                        engs[ei % 3].dma_start(
                            out=dst[32 * g:32 * (g + 1), ua - u0t:ub - u0t,
                                    :, :],
                            in_=src[:, 0:ub - ua, :, :],
                        )
                        ei += 1
                        # lower: u in [u0t, 31+g), a = 31+g-u descending
                        la, lb = u0t, 31 + g
                        amax = 31 + g - u0t
                        engs[ei % 3].dma_start(
                            out=dst[32 * g:32 * (g + 1), 0:lb - la, :, :],
                            in_=bass.AP(tensor=src.tensor,
                                        offset=src.offset + amax * 256,
                                        ap=[src.ap[0], [-256, lb - la],
                                            [1, 256]]),
                        )
                        ei += 1

            # ---------------- BN affine rows ---------------------------
            g2b = constp.tile([128, DOUT], F32)
            b2b = constp.tile([128, DOUT], F32)
            tmpb = constp.tile([128, DOUT], F32)
            nc.sync.dma_start(
                out=g2b, in_=bass.AP(tensor=gam.tensor, offset=0,
                                     ap=[[0, 128], [1, DOUT]]))
            nc.sync.dma_start(
                out=b2b, in_=bass.AP(tensor=bet.tensor, offset=0,
                                     ap=[[0, 128], [1, DOUT]]))
            nc.sync.dma_start(
                out=tmpb, in_=bass.AP(tensor=bo.tensor, offset=0,
                                      ap=[[0, 128], [1, DOUT]]))
            wstg_o = xinp.tile([128, 4, 256], F32, tag="wstgo", bufs=1)
            nc.sync.dma_start(
                out=wstg_o,
                in_=bass.AP(tensor=wo.tensor, offset=0,
                            ap=[[256, 128], [128 * 256, 4], [1, 256]]))
            nc.scalar.mul(g2b, g2b, BN_C)
            nc.vector.tensor_mul(tmpb, tmpb, g2b)
            nc.vector.tensor_add(b2b, b2b, tmpb)

            # ---------------- weights (convert to bf16) ----------------
            wq_sb = constp.tile([128, 2, 256], BF16)
            wk_sb = constp.tile([128, 2, 256], BF16)
            wv_sb = constp.tile([128, 2, 512], BF16)
            wo_sb = constp.tile([128, 4, 256], BF16)
            nc.scalar.mul(wq_sb, wstg_q, SCALE)   # fold softmax scale
            nc.gpsimd.tensor_copy(wk_sb, wstg_k)
            nc.gpsimd.tensor_copy(wv_sb, wstg_v)
            # fold BN gamma*c into Wo columns (broadcast g2b over kt)
            nc.vector.tensor_mul(
                wo_sb, wstg_o,
                bass.AP(tensor=g2b.tensor, offset=g2b.offset,
                        ap=[g2b.ap[0], [0, 4], [1, 256]]))

            def _copy(i, dst, src):
                e = (nc.scalar.copy, nc.vector.tensor_copy)[i % 2]
                e(dst, src)

            # ---------------- phase A: x -> xT (bf16) ------------------
            xT = bigp.tile([128, 2, N], BF16)
            for nt in range(8):
                for dt in range(2):
                    pst = ps1p.tile([128, 512], F32, tag="ps1")
                    nc.tensor.transpose(pst[:, 0:128],
                                        xa[:, nt, 128 * dt:128 * (dt + 1)],
                                        identf)
                    _copy(nt + dt, xT[:, dt, 128 * nt:128 * (nt + 1)],
                          pst[:, 0:128])

            # ---------------- phase B: QKV proj ------------------------
            qT = bigp.tile([128, 2, N], BF16)
            kT = bigp.tile([128, 2, N], BF16)
            va = bigp.tile([128, 8, H, 65], BF16)
            nc.scalar.activation(va[:, :, :, 64:65], identb[:, 0:64],
                                 AF.Copy, bias=1.0, scale=0.0)
            ci_ = 0

            def _qk(mt):
                nonlocal ci_
                for dst_sb, w_sb in ((qT, wq_sb), (kT, wk_sb)):
                    for ic in range(2):
                        ps = ps1p.tile([128, 512], F32, tag="ps1")
                        for kt in range(2):
                            nc.tensor.matmul(
                                ps,
                                w_sb[:, kt, 128 * mt:128 * (mt + 1)],
                                xT[:, kt, 512 * ic:512 * (ic + 1)],
                                start=(kt == 0), stop=(kt == 1),
                            )
                        _copy(ci_, dst_sb[:, mt, 512 * ic:512 * (ic + 1)], ps)
                        ci_ += 1

            _qk(0)   # heads 0-3 only need mt 0 - lets phase C start early
            for jt in range(8):
                ps = ps1p.tile([128, 512], F32, tag="ps1")
                for kt in range(2):
                    nc.tensor.matmul(
                        ps,
                        xT[:, kt, 128 * jt:128 * (jt + 1)],
                        wv_sb[:, kt, :],
                        start=(kt == 0), stop=(kt == 1),
                    )
                psr = ps.rearrange("p (h v) -> p h v", v=64)
                _copy(ci_, va[:, jt, :, 0:64], psr)
                ci_ += 1
            _qk(1)

            # ---------------- phase C: attention -----------------------
            # g_all[i-part, it, h, dv] collects gelu(attn/den), bf16
            g_all = bigp.tile([128, 8, H, DV], BF16)
            gtt = bigp.tile([128, 4, 8, 128], BF16)

            def _tail_transpose(blk):
                # gtt[:, blk, it, :] = g_all[:, it, 2blk:2blk+2, :]^T
                for it in range(8):
                    pst = ps1p.tile([128, 512], F32, tag="ps1")
                    pstb = pst[:, 0:64].bitcast(BF16)
                    nc.tensor.transpose(
                        pstb,
                        g_all[:, it, 2 * blk:2 * blk + 2, :], identb)
                    nc.vector.tensor_copy(gtt[:, blk, it, :], pstb)

            def _attnv_group(h, it, esbbs, pos):
                for jt in range(8):
                    nc.tensor.matmul(
                        pos[it // 4][:, it % 4, :],
                        esbbs[jt][:, 128 * it:128 * (it + 1)],
                        va[:, jt, h, :],
                        start=(jt == 0), stop=(jt == 7),
                    )

            def _norm_gelu(h, half, pos, rcp):
                den = pos[half][:, :, 64:65]
                nc.vector.reciprocal(
                    rcp[:, 4 * half:4 * half + 4],
                    bass.AP(tensor=den.tensor, offset=den.offset,
                            ap=[den.ap[0], [65, 4]]))
                for it in range(4 * half, 4 * half + 4):
                    nc.scalar.activation(
                        g_all[:, it, h, :],
                        pos[half][:, it % 4, 0:64],
                        AF.Gelu, scale=rcp[:, it:it + 1])

            # attnV/normalize are software-pipelined one head behind the
            # score/exp stream so the attnV matmul groups fill PE stalls
            # between score matmuls (keeps PE p-state ramped).
            prev = None
            for h in range(H):
                mtk = h // 4
                pb = 32 * (h % 4)
                po0 = pop.tile([128, 4, 65], F32, tag="po0")
                po1 = pop.tile([128, 4, 65], F32, tag="po1")
                pos = (po0, po1)
                esbbs = [None] * 8
                for step, jt in enumerate((0, 4, 1, 5, 2, 6, 3, 7)):
                    u0 = 31 - 4 * jt
                    ps = ps2p.tile([128, 1024], F32, tag="st")
                    for ic in range(2):
                        nc.tensor.matmul(
                            ps[:, 512 * ic:512 * (ic + 1)],
                            kT[pb:pb + 32, mtk, 128 * jt:128 * (jt + 1)],
                            qT[pb:pb + 32, mtk, 512 * ic:512 * (ic + 1)],
                            start=True, stop=True,
                            tile_position=(pb, 0),
                        )
                    if jt < N_A:
                        # exact exp on ACT, bias multiply (DVE 2x / Pool)
                        es = expp.tile([128, 1024], BF16, tag="es")
                        nc.scalar.activation(es, ps, AF.Exp)
                        esb = esbp.tile([128, 1024], BF16, tag="esb", bufs=8)
                        msl = ms[:, u0 - MS_U0:u0 - MS_U0 + 32, h, :]
                        msr = bass.AP(tensor=msl.tensor,
                                      offset=msl.offset + 31,
                                      ap=[msl.ap[0], msl.ap[1], [-1, 32]])
                        beng = nc.gpsimd if jt in POOL_BIAS_JT else nc.vector
                        beng.tensor_tensor(esb, es, msr, ALU.mult)
                        esbbs[jt] = esb
                    else:
                        # fused Schraudolph exp+bias on DVE:
                        # bits_i16 = ps*A + (b/scale*A + B) -> bitcast bf16
                        esb = esbp.tile([128, 1024], I16, tag="esbi", bufs=8)
                        msl = msa[:, u0 - MSA_U0:u0 - MSA_U0 + 32, h, :]
                        msr = bass.AP(tensor=msl.tensor,
                                      offset=msl.offset + 31,
                                      ap=[msl.ap[0], msl.ap[1], [-1, 32]])
                        nc.vector.scalar_tensor_tensor(
                            esb, ps, SCH_A, msr, ALU.mult, ALU.add)
                        esbbs[jt] = esb.bitcast(BF16)
                    if prev is not None:
                        _attnv_group(h - 1, step, prev[0], prev[1])
                        if step == 3:
                            _norm_gelu(h - 1, 0, prev[1], prev[2])
                        elif step == 7:
                            _norm_gelu(h - 1, 1, prev[1], prev[2])
                            if h % 2 == 0:
                                _tail_transpose(h // 2 - 1)
                rcp = smallp.tile([128, 8], F32, tag="rcp", bufs=2)
                prev = (esbbs, pos, rcp)
            # drain last head: attnV groups + normalize + transposes
            for it in range(8):
                _attnv_group(7, it, prev[0], prev[1])
                if it == 3:
                    _norm_gelu(7, 0, prev[1], prev[2])
            _norm_gelu(7, 1, prev[1], prev[2])
            _tail_transpose(3)

            # ---------------- tail: out proj ---------------------------
            for it in range(8):
                ps = ps1p.tile([128, 512], F32, tag="ps1")
                for kt in range(4):
                    nc.tensor.matmul(
                        ps[:, 0:256],
                        gtt[:, kt, it, :],
                        wo_sb[:, kt, :],
                        start=(kt == 0), stop=(kt == 3),
                    )
                yt = youtp.tile([128, DOUT], F32, tag="yt")
                nc.vector.tensor_add(yt, ps[:, 0:256], b2b)
                eng = nc.sync if it % 2 == 0 else nc.scalar
                eng.dma_start(out=out[128 * it:128 * (it + 1), :], in_=yt)

    _split_excess_waits(nc)
    return nc


def _split_excess_waits(nc):
    """walrus rejects >1 sem-wait per instruction ("Too many sync wait
    commands"); unroll extras into a chain of single-wait same-engine
    NoOps directly before the instruction."""
    ctr = 0
    for fn in nc.m.functions:
        for blk in fn.blocks:
            out = []
            for inst in blk.instructions:
                si = inst.sync_info
                if si is not None and len(si.on_wait) > 1:
                    for w in si.on_wait[:-1]:
                        nop = mybir.InstNoOp(name=f"waitnop-{ctr}")
                        ctr += 1
                        nop.engine = inst.engine
                        nop.sync_info = mybir.SyncInfo(
                            on_wait=[w], on_update=[])
                        out.append(nop)
                    inst.sync_info = mybir.SyncInfo(
                        on_wait=[si.on_wait[-1]], on_update=list(si.on_update))
                out.append(inst)
            blk.instructions = out


_NC_CACHE = None


def kernel(**inputs) -> np.ndarray:
    global _NC_CACHE
    x = np.ascontiguousarray(inputs["x"], dtype=np.float32)        # (8,32,32,256)
    shared = {
        "wq": np.ascontiguousarray(inputs["Wq"], dtype=np.float32),
        "wk": np.ascontiguousarray(inputs["Wk"], dtype=np.float32),
        "wv": np.ascontiguousarray(inputs["Wv"], dtype=np.float32),
        "wo": np.ascontiguousarray(inputs["Wo"], dtype=np.float32),
        "pe": np.ascontiguousarray(inputs["pos_emb"], dtype=np.float32),
        "bo": np.ascontiguousarray(inputs["bo"], dtype=np.float32),
        "gam": np.ascontiguousarray(inputs["gamma"], dtype=np.float32),
        "bet": np.ascontiguousarray(inputs["beta"], dtype=np.float32),
    }
    in_maps = []
    for c in range(NCORES):
        m = dict(shared)
        m["x"] = np.ascontiguousarray(x[c].reshape(N, D))
        in_maps.append(m)

    if _NC_CACHE is None:
        _NC_CACHE = build_nc()
    res = run_bass_kernel_spmd(_NC_CACHE, in_maps, core_ids=list(range(NCORES)))
    outs = [res.results[c]["out"].reshape(FM, FM, DOUT) for c in range(NCORES)]
    return np.stack(outs, axis=0)


if __name__ == "__main__":
    build_nc()
    print("build ok")


# revision 33
# speedup vs baseline: 1.0594x; 1.0326x over previous
"""Trainium2 Bass kernel for nn_Attention_89172110999574.

Strategy (8 NeuronCores, data parallel - 1 batch element per core), v4:
  - All matmul operands bf16 (1 cyc/row on PE); PSUM stays f32.
  - Scores computed TRANSPOSED (ST[j,i] = k_j . q_i); softmax scale folded
    into Wq at load.
  - Relative-position bias is block-Toeplitz; two per-head strip tables are
    built once via a handful of large-elem DMAs (DRAM bounce for the
    partition reshuffle):
      * ms  (bf16):  exp(bias) factors for the exact-exp path (jt 0-3)
      * msa (int16): bias*A + B Schraudolph addends for the fast path
    exp(s+b) = exp(s)*exp(b) on the A path (ACT exp + DVE/Pool
    tensor_tensor multiply at 2x bf16 rate), and
    bf16_bits(exp(s+b)) ~= s*A + (b*A + B) on the V path - a single DVE
    op (Schraudolph's trick in bf16, max ~3% weight ripple, cancels in
    the softmax ratio; validated 4e-3 end-to-end).
  - attn@V uses exp-scores as stationary -> [i, dv+1] tiles with the
    softmax denominator in the last column.  Each PSUM accumulation group
    runs as 8 consecutive matmuls (interleaved groups in one bank
    misaccumulate on hardware).
  - normalize+gelu fused: ACT Gelu with per-partition scale = 1/den
    (reciprocals batched 4-wide on DVE) reading straight from PSUM.
  - BatchNorm affine folded into Wo/bias rows; per-i-tile tail:
    transpose via PE, final contraction, store.
"""

import os
import sys

import numpy as np

for _p in ("/opt/trn_rl_repo", "/root/.axon_site/_ro/trn_rl_repo"):
    if os.path.isdir(_p) and _p not in sys.path:
        sys.path.insert(0, _p)

import concourse.bass as bass
import concourse.tile as tile
from concourse import mybir
from concourse.bass_utils import run_bass_kernel_spmd
from concourse.masks import make_identity

N = 1024          # tokens per batch (32*32)
D = 256           # model dim
H = 8             # heads
DK = 32           # head dim (qk)
DV = 64           # head dim (v)
DOUT = 256        # output dim
NCORES = 8
FM = 32           # fmap
SCALE = float(DK) ** -0.5          # 1/sqrt(32)
BN_C = float(1.0 / np.sqrt(1.0 + 1e-5))
SCH_A = float(2 ** 7 / np.log(2.0))        # Schraudolph bf16 scale
SCH_B = float(127 * 2 ** 7 - 7.4 + 0.5)    # bias - minimax + trunc comp
F32 = mybir.dt.float32
BF16 = mybir.dt.bfloat16
I16 = mybir.dt.int16
AF = mybir.ActivationFunctionType
ALU = mybir.AluOpType

# jt 0..3: exact exp on ACT (+ bias multiply); jt 4..7: Schraudolph on DVE
N_A = 4
POOL_BIAS_JT = (0, 1, 2)       # A-tiles whose bias multiply runs on Pool
# strip-table u ranges (u0 = 31-4*jt, slice [u0, u0+32))
MS_U0, MS_UN = 19, 44          # A tiles: u in [19, 62]
MSA_U0, MSA_UN = 3, 44         # V tiles: u in [3, 46]


def build_nc():
    nc = bass.Bass("TRN2", target_bir_lowering=False, debug=False)

    x = nc.dram_tensor("x", [N, D], F32, kind="ExternalInput").ap()
    wq = nc.dram_tensor("wq", [D, H * DK], F32, kind="ExternalInput").ap()
    wk = nc.dram_tensor("wk", [D, H * DK], F32, kind="ExternalInput").ap()
    wv = nc.dram_tensor("wv", [D, H * DV], F32, kind="ExternalInput").ap()
    wo = nc.dram_tensor("wo", [H * DV, DOUT], F32, kind="ExternalInput").ap()
    pe = nc.dram_tensor("pe", [N, H], F32, kind="ExternalInput").ap()
    bo = nc.dram_tensor("bo", [DOUT], F32, kind="ExternalInput").ap()
    gam = nc.dram_tensor("gam", [DOUT], F32, kind="ExternalInput").ap()
    bet = nc.dram_tensor("bet", [DOUT], F32, kind="ExternalInput").ap()
    out = nc.dram_tensor("out", [N, DOUT], F32, kind="ExternalOutput").ap()

    # DRAM bounce buffers for the strip tables: [a, h, s] flipped rows
    wrowd = nc.dram_tensor("wrowd", [32, 8, 63], BF16).ap()
    wrowda = nc.dram_tensor("wrowda", [32, 8, 63], I16).ap()

    with tile.TileContext(nc) as tc:
        with (
            tc.tile_pool(name="const", bufs=1) as constp,
            tc.tile_pool(name="big", bufs=1) as bigp,
            tc.tile_pool(name="xin", bufs=3) as xinp,
            tc.tile_pool(name="exps", bufs=4) as expp,
            tc.tile_pool(name="esb", bufs=1) as esbp,
            tc.tile_pool(name="small", bufs=2) as smallp,
            tc.tile_pool(name="yout", bufs=3) as youtp,
            tc.tile_pool(name="ps1", bufs=2, space="PSUM") as ps1p,
            tc.tile_pool(name="st", bufs=2, space="PSUM") as ps2p,
            tc.tile_pool(name="po", bufs=1, space="PSUM") as pop,
        ):
            # ------------- input / weight DMAs first (fabric order) -----
            xa = bigp.tile([128, 8, 256], F32)
            for c in range(2):
                nc.sync.dma_start(
                    out=xa[:, 4 * c:4 * (c + 1), :],
                    in_=bass.AP(tensor=x.tensor, offset=4 * c * 128 * 256,
                                ap=[[256, 128], [128 * 256, 4], [1, 256]]))
            wstg_q = xinp.tile([128, 2, 256], F32, tag="wstg2", bufs=2)
            nc.scalar.dma_start(
                out=wstg_q,
                in_=bass.AP(tensor=wq.tensor, offset=0,
                            ap=[[256, 128], [128 * 256, 2], [1, 256]]))
            wstg_k = xinp.tile([128, 2, 256], F32, tag="wstg2", bufs=2)
            nc.scalar.dma_start(
                out=wstg_k,
                in_=bass.AP(tensor=wk.tensor, offset=0,
                            ap=[[256, 128], [128 * 256, 2], [1, 256]]))
            wstg_v = xinp.tile([128, 2, 512], F32, tag="wstgv", bufs=1)
            nc.scalar.dma_start(
                out=wstg_v,
                in_=bass.AP(tensor=wv.tensor, offset=0,
                            ap=[[512, 128], [128 * 512, 2], [1, 512]]))
            e_sb = smallp.tile([32, 32, 8], F32, tag="e_sb")
            nc.sync.dma_start(
                out=e_sb,
                in_=bass.AP(tensor=pe.tensor, offset=0,
                            ap=[[32 * H, 32], [H, 32], [1, 32 * H // 32]]),
            )

            # ---------------- constants -------------------------------
            identb = constp.tile([128, 128], BF16)
            make_identity(nc, identb)
            identf = constp.tile([128, 128], F32)
            nc.gpsimd.tensor_copy(identf, identb)

            # ---------------- strip tables ----------------------------
            # 2a) mult table: ee = exp(pe/scale) bf16
            ee = smallp.tile([32, 32, 8], BF16, tag="ee")
            nc.scalar.activation(ee, e_sb, AF.Exp, scale=1.0 / SCALE)
            # 2b) additive Schraudolph table: eea = pe/scale*A + B int16
            eea = smallp.tile([32, 32, 8], I16, tag="eea")
            nc.scalar.activation(eea, e_sb, AF.Copy,
                                 scale=SCH_A / SCALE, bias=SCH_B)
            # 3) s-flip both: wrow[a, h, s] = tab[a, |s-31|, h]
            wrow = smallp.tile([32, 8, 63], BF16, tag="wrow")
            wrowa = smallp.tile([32, 8, 63], I16, tag="wrowa")
            for wdst, wsrc in ((wrow, ee), (wrowa, eea)):
                nc.gpsimd.tensor_copy(
                    wdst[:, :, 0:31],
                    bass.AP(tensor=wsrc.tensor, offset=wsrc.offset + 31 * 8,
                            ap=[wsrc.ap[0], [1, 8], [-8, 31]]),
                )
                nc.gpsimd.tensor_copy(
                    wdst[:, :, 31:63],
                    bass.AP(tensor=wsrc.tensor, offset=wsrc.offset,
                            ap=[wsrc.ap[0], [1, 8], [8, 32]]),
                )
            # 4) DRAM bounce + gather (positive strides; wrow palindromic
            #    in s, so the gathered ci axis comes out reversed)
            nc.sync.dma_start(out=wrowd, in_=wrow)
            nc.scalar.dma_start(out=wrowda, in_=wrowa)
            # ms[(g,cj), u-U0, h, ci'] = tab_h[|u-31-g|, |ci-cj|]
            ms = bigp.tile([128, MS_UN, H, 32], BF16)
            msa = bigp.tile([128, MSA_UN, H, 32], I16)
            with tc.tile_pool(name="uw", bufs=1) as uwp:
                uwsb = uwp.tile([32, 32, H, 32], BF16)
                uwsba = uwp.tile([32, 32, H, 32], I16)
                nc.sync.dma_start(
                    out=uwsb,
                    in_=bass.AP(tensor=wrowd.tensor, offset=0,
                                ap=[[1, 32], [504, 32], [63, 8], [1, 32]]),
                )
                nc.scalar.dma_start(
                    out=uwsba,
                    in_=bass.AP(tensor=wrowda.tensor, offset=0,
                                ap=[[1, 32], [504, 32], [63, 8], [1, 32]]),
                )
                # 5) u-expansion: dst[(g,cj), u] = uwsb[cj, |u-31-g|]
                engs = (nc.sync, nc.scalar, nc.gpsimd)
                ei = 0
                for src, dst, u0t, un in ((uwsb, ms, MS_U0, MS_UN),
                                          (uwsba, msa, MSA_U0, MSA_UN)):
                    for g in range(4):
                        # upper: u in [31+g, u0t+un), a = u-31-g ascending
                        ua, ub = 31 + g, u0t + un
                        engs[ei % 3].dma_start(
                            out=dst[32 * g:32 * (g + 1), ua - u0t:ub - u0t,
                                    :, :],
                            in_=src[:, 0:ub - ua, :, :],
                        )
                        ei += 1
                        # lower: u in [u0t, 31+g), a = 31+g-u descending
                        la, lb = u0t, 31 + g
                        amax = 31 + g - u0t
                        engs[ei % 3].dma_start(
                            out=dst[32 * g:32 * (g + 1), 0:lb - la, :, :],
                            in_=bass.AP(tensor=src.tensor,
                                        offset=src.offset + amax * 256,
                                        ap=[src.ap[0], [-256, lb - la],
                                            [1, 256]]),
                        )
                        ei += 1

            # ---------------- BN affine rows ---------------------------
            g2b = constp.tile([128, DOUT], F32)
            b2b = constp.tile([128, DOUT], F32)
            tmpb = constp.tile([128, DOUT], F32)
            nc.sync.dma_start(
                out=g2b, in_=bass.AP(tensor=gam.tensor, offset=0,
                                     ap=[[0, 128], [1, DOUT]]))
            nc.sync.dma_start(
                out=b2b, in_=bass.AP(tensor=bet.tensor, offset=0,
                                     ap=[[0, 128], [1, DOUT]]))
            nc.sync.dma_start(
                out=tmpb, in_=bass.AP(tensor=bo.tensor, offset=0,
                                      ap=[[0, 128], [1, DOUT]]))
            wstg_o = xinp.tile([128, 4, 256], F32, tag="wstgo", bufs=1)
            nc.sync.dma_start(
                out=wstg_o,
                in_=bass.AP(tensor=wo.tensor, offset=0,
                            ap=[[256, 128], [128 * 256, 4], [1, 256]]))
            nc.scalar.mul(g2b, g2b, BN_C)
            nc.vector.tensor_mul(tmpb, tmpb, g2b)
            nc.vector.tensor_add(b2b, b2b, tmpb)

            # ---------------- weights (convert to bf16) ----------------
            wq_sb = constp.tile([128, 2, 256], BF16)
            wk_sb = constp.tile([128, 2, 256], BF16)
            wv_sb = constp.tile([128, 2, 512], BF16)
            wo_sb = constp.tile([128, 4, 256], BF16)
            nc.scalar.mul(wq_sb, wstg_q, SCALE)   # fold softmax scale
            nc.gpsimd.tensor_copy(wk_sb, wstg_k)
            nc.gpsimd.tensor_copy(wv_sb, wstg_v)
            # fold BN gamma*c into Wo columns (broadcast g2b over kt)
            nc.vector.tensor_mul(
                wo_sb, wstg_o,
                bass.AP(tensor=g2b.tensor, offset=g2b.offset,
                        ap=[g2b.ap[0], [0, 4], [1, 256]]))

            def _copy(i, dst, src):
                e = (nc.scalar.copy, nc.vector.tensor_copy)[i % 2]
                e(dst, src)

            # ---------------- phase A: x -> xT (bf16) ------------------
            xT = bigp.tile([128, 2, N], BF16)
            for nt in range(8):
                pst = ps1p.tile([128, 512], F32, tag="ps1")
                for dt in range(2):
                    nc.tensor.transpose(pst[:, 128 * dt:128 * (dt + 1)],
                                        xa[:, nt, 128 * dt:128 * (dt + 1)],
                                        identf)
                _copy(nt, xT[:, 0, 128 * nt:128 * (nt + 1)],
                      pst[:, 0:128])
                _copy(nt + 1, xT[:, 1, 128 * nt:128 * (nt + 1)],
                      pst[:, 128:256])

            # ---------------- phase B: QKV proj ------------------------
            qT = bigp.tile([128, 2, N], BF16)
            kT = bigp.tile([128, 2, N], BF16)
            va = bigp.tile([128, 8, H, 65], BF16)
            nc.scalar.activation(va[:, :, :, 64:65], identb[:, 0:64],
                                 AF.Copy, bias=1.0, scale=0.0)
            ci_ = 0

            def _qk(mt):
                nonlocal ci_
                for dst_sb, w_sb in ((qT, wq_sb), (kT, wk_sb)):
                    for ic in range(2):
                        ps = ps1p.tile([128, 512], F32, tag="ps1")
                        for kt in range(2):
                            nc.tensor.matmul(
                                ps,
                                w_sb[:, kt, 128 * mt:128 * (mt + 1)],
                                xT[:, kt, 512 * ic:512 * (ic + 1)],
                                start=(kt == 0), stop=(kt == 1),
                            )
                        _copy(ci_, dst_sb[:, mt, 512 * ic:512 * (ic + 1)], ps)
                        ci_ += 1

            _qk(0)   # heads 0-3 only need mt 0 - lets phase C start early
            for jt in range(8):
                ps = ps1p.tile([128, 512], F32, tag="ps1")
                for kt in range(2):
                    nc.tensor.matmul(
                        ps,
                        xT[:, kt, 128 * jt:128 * (jt + 1)],
                        wv_sb[:, kt, :],
                        start=(kt == 0), stop=(kt == 1),
                    )
                psr = ps.rearrange("p (h v) -> p h v", v=64)
                _copy(ci_, va[:, jt, :, 0:64], psr)
                ci_ += 1
            _qk(1)

            # ---------------- phase C: attention -----------------------
            # g_all[i-part, it, h, dv] collects gelu(attn/den), bf16
            g_all = bigp.tile([128, 8, H, DV], BF16)
            gtt = bigp.tile([128, 4, 8, 128], BF16)

            def _tail_transpose(blk):
                # gtt[:, blk, it, :] = g_all[:, it, 2blk:2blk+2, :]^T
                for it in range(8):
                    pst = ps1p.tile([128, 512], F32, tag="ps1")
                    pstb = pst[:, 0:64].bitcast(BF16)
                    nc.tensor.transpose(
                        pstb,
                        g_all[:, it, 2 * blk:2 * blk + 2, :], identb)
                    nc.vector.tensor_copy(gtt[:, blk, it, :], pstb)

            def _attnv_group(h, it, esbbs, pos):
                for jt in range(8):
                    nc.tensor.matmul(
                        pos[it // 4][:, it % 4, :],
                        esbbs[jt][:, 128 * it:128 * (it + 1)],
                        va[:, jt, h, :],
                        start=(jt == 0), stop=(jt == 7),
                    )

            def _norm_gelu(h, half, pos, rcp):
                den = pos[half][:, :, 64:65]
                nc.vector.reciprocal(
                    rcp[:, 4 * half:4 * half + 4],
                    bass.AP(tensor=den.tensor, offset=den.offset,
                            ap=[den.ap[0], [65, 4]]))
                for it in range(4 * half, 4 * half + 4):
                    nc.scalar.activation(
                        g_all[:, it, h, :],
                        pos[half][:, it % 4, 0:64],
                        AF.Gelu, scale=rcp[:, it:it + 1])

            # attnV/normalize are software-pipelined one head behind the
            # score/exp stream so the attnV matmul groups fill PE stalls
            # between score matmuls (keeps PE p-state ramped).
            prev = None
            for h in range(H):
                mtk = h // 4
                pb = 32 * (h % 4)
                po0 = pop.tile([128, 4, 65], F32, tag="po0")
                po1 = pop.tile([128, 4, 65], F32, tag="po1")
                pos = (po0, po1)
                esbbs = [None] * 8
                for step, jt in enumerate((0, 4, 1, 5, 2, 6, 3, 7)):
                    u0 = 31 - 4 * jt
                    ps = ps2p.tile([128, 1024], F32, tag="st")
                    for ic in range(2):
                        nc.tensor.matmul(
                            ps[:, 512 * ic:512 * (ic + 1)],
                            kT[pb:pb + 32, mtk, 128 * jt:128 * (jt + 1)],
                            qT[pb:pb + 32, mtk, 512 * ic:512 * (ic + 1)],
                            start=True, stop=True,
                            tile_position=(pb, 0),
                        )
                    if jt < N_A:
                        # exact exp on ACT, bias multiply (DVE 2x / Pool)
                        es = expp.tile([128, 1024], BF16, tag="es")
                        nc.scalar.activation(es, ps, AF.Exp)
                        esb = esbp.tile([128, 1024], BF16, tag="esb", bufs=8)
                        msl = ms[:, u0 - MS_U0:u0 - MS_U0 + 32, h, :]
                        msr = bass.AP(tensor=msl.tensor,
                                      offset=msl.offset + 31,
                                      ap=[msl.ap[0], msl.ap[1], [-1, 32]])
                        beng = nc.gpsimd if jt in POOL_BIAS_JT else nc.vector
                        beng.tensor_tensor(esb, es, msr, ALU.mult)
                        esbbs[jt] = esb
                    else:
                        # fused Schraudolph exp+bias on DVE:
                        # bits_i16 = ps*A + (b/scale*A + B) -> bitcast bf16
                        esb = esbp.tile([128, 1024], I16, tag="esbi", bufs=8)
                        msl = msa[:, u0 - MSA_U0:u0 - MSA_U0 + 32, h, :]
                        msr = bass.AP(tensor=msl.tensor,
                                      offset=msl.offset + 31,
                                      ap=[msl.ap[0], msl.ap[1], [-1, 32]])
                        nc.vector.scalar_tensor_tensor(
                            esb, ps, SCH_A, msr, ALU.mult, ALU.add)
                        esbbs[jt] = esb.bitcast(BF16)
                rcp = smallp.tile([128, 8], F32, tag="rcp", bufs=2)
                for half in range(2):
                    for it in range(4 * half, 4 * half + 4):
                        _attnv_group(h, it, esbbs, pos)
                    if half == 0:
                        _norm_gelu(h, 0, pos, rcp)
                _norm_gelu(h, 1, pos, rcp)
                if h % 2 == 1:
                    _tail_transpose(h // 2)
                prev = None

            # ---------------- tail: out proj ---------------------------
            for it in range(8):
                ps = ps1p.tile([128, 512], F32, tag="ps1")
                for kt in range(4):
                    nc.tensor.matmul(
                        ps[:, 0:256],
                        gtt[:, kt, it, :],
                        wo_sb[:, kt, :],
                        start=(kt == 0), stop=(kt == 3),
                    )
                yt = youtp.tile([128, DOUT], F32, tag="yt")
                nc.vector.tensor_add(yt, ps[:, 0:256], b2b)
                eng = nc.sync if it % 2 == 0 else nc.scalar
                eng.dma_start(out=out[128 * it:128 * (it + 1), :], in_=yt)

    _split_excess_waits(nc)
    return nc


def _split_excess_waits(nc):
    """walrus rejects >1 sem-wait per instruction ("Too many sync wait
    commands"); unroll extras into a chain of single-wait same-engine
    NoOps directly before the instruction."""
    ctr = 0
    for fn in nc.m.functions:
        for blk in fn.blocks:
            out = []
            for inst in blk.instructions:
                si = inst.sync_info
                if si is not None and len(si.on_wait) > 1:
                    for w in si.on_wait[:-1]:
                        nop = mybir.InstNoOp(name=f"waitnop-{ctr}")
                        ctr += 1
                        nop.engine = inst.engine
                        nop.sync_info = mybir.SyncInfo(
                            on_wait=[w], on_update=[])
                        out.append(nop)
                    inst.sync_info = mybir.SyncInfo(
                        on_wait=[si.on_wait[-1]], on_update=list(si.on_update))
                out.append(inst)
            blk.instructions = out


_NC_CACHE = None


def kernel(**inputs) -> np.ndarray:
    global _NC_CACHE
    x = np.ascontiguousarray(inputs["x"], dtype=np.float32)        # (8,32,32,256)
    shared = {
        "wq": np.ascontiguousarray(inputs["Wq"], dtype=np.float32),
        "wk": np.ascontiguousarray(inputs["Wk"], dtype=np.float32),
        "wv": np.ascontiguousarray(inputs["Wv"], dtype=np.float32),
        "wo": np.ascontiguousarray(inputs["Wo"], dtype=np.float32),
        "pe": np.ascontiguousarray(inputs["pos_emb"], dtype=np.float32),
        "bo": np.ascontiguousarray(inputs["bo"], dtype=np.float32),
        "gam": np.ascontiguousarray(inputs["gamma"], dtype=np.float32),
        "bet": np.ascontiguousarray(inputs["beta"], dtype=np.float32),
    }
    in_maps = []
    for c in range(NCORES):
        m = dict(shared)
        m["x"] = np.ascontiguousarray(x[c].reshape(N, D))
        in_maps.append(m)

    if _NC_CACHE is None:
        _NC_CACHE = build_nc()
    res = run_bass_kernel_spmd(_NC_CACHE, in_maps, core_ids=list(range(NCORES)))
    outs = [res.results[c]["out"].reshape(FM, FM, DOUT) for c in range(NCORES)]
    return np.stack(outs, axis=0)


if __name__ == "__main__":
    build_nc()
    print("build ok")
